# revision 4
# baseline (speedup 1.0000x reference)
"""BitLinear (4-bit activation quant + ternary weight) Trainium2 kernel.

Full computation:
    xq  = round(clip(x / max_abs(x, row) * 7)) * max_abs / 7      (per-row 4-bit quant)
    wq  = sign_thresholded(w) * mean_abs(w, row)                   (ternary weight)
    out = xq @ wq.T + bias

Strategy (8 NeuronCores, data-parallel over rows of x):
  - Shard x rows 8 ways; replicate weight.
  - x and weight ship to the device as f16 (half the HBM read traffic; the
    quant decisions from f16 inputs keep the end-to-end rel err ~9.6e-3,
    within the 2e-2 gate). Output is exact f32 from integer PE accumulation.
  - On-chip, the matmul runs on exact small integers in fp8 (q in [-8,7],
    sign in {-1,0,1}) with DoubleRow perf mode; the row scale (max_abs/7) and
    column scale (alpha) are applied to the f32 PSUM output in one fused
    scalar_tensor_tensor eviction on the Pool engine.
  - Rounding uses the +1.5*2^23 magic-number trick == round-half-even,
    matching jnp.round bit-for-bit on the f16-valued inputs.
  - Engine balance per 128-row s-tile (DMA is the global bottleneck at
    ~2.37us/s-tile): DVE does absmax+scales (~1.4us), Pool does the magic
    multiply-add and the PSUM->out eviction (~1.9us), ACT does the
    quantized-transpose eviction (~1.4us), PE does transposes+matmuls
    (~1.7us at full clock).
"""

import os
import sys

os.environ.setdefault("MYCRO_LOCAL_CACHE", "1")

for _p in ("/opt/trn_rl_repo", "/root/.axon_site/_ro/trn_rl_repo"):
    if os.path.isdir(_p) and _p not in sys.path:
        sys.path.insert(0, _p)

import numpy as np

N_CORES = 8
S_SHARD = 4096  # rows of x per core (8*4096 total / 8 cores)
IN_F = 1024
OUT_F = 1024
P = 128  # partitions
N_STILES = S_SHARD // P  # 32
N_KTILES = IN_F // P  # 8
N_OTILES = OUT_F // P  # 8
MM_N = 512  # matmul moving free dim (one PSUM bank of f32)
N_OHALF = OUT_F // MM_N  # 2

MAGIC = 12582912.0  # 1.5 * 2**23: float32 add == round-to-nearest-even
EPS = 1e-06

_prog_cache = {}


def _build_program(with_bias: bool):
    import concourse.bass as bass
    import concourse.mybir as mybir
    import concourse.tile as tile
    from concourse import bacc, bass_isa
    from concourse.masks import make_identity

    f32 = mybir.dt.float32
    f16 = mybir.dt.float16
    bf16 = mybir.dt.bfloat16
    f8 = mybir.dt.float8e4
    Alu = mybir.AluOpType
    Act = mybir.ActivationFunctionType

    nc = bacc.Bacc("TRN2", target_bir_lowering=False, debug=False)

    x_in = nc.dram_tensor("x_shard", [S_SHARD, IN_F], f16, kind="ExternalInput")
    w_in = nc.dram_tensor("weight", [OUT_F, IN_F], f16, kind="ExternalInput")
    if with_bias:
        b_in = nc.dram_tensor("bias", [OUT_F], f32, kind="ExternalInput")
    out_d = nc.dram_tensor("out", [S_SHARD, OUT_F], f32, kind="ExternalOutput")

    with tile.TileContext(nc) as tc:
        from contextlib import ExitStack as _ES

        _wstack = _ES()
        with (
            tc.tile_pool(name="singles", bufs=1) as singles,
            tc.tile_pool(name="wtmp", bufs=2) as wtmp,
            tc.tile_pool(name="signp", bufs=2) as signp,
            tc.tile_pool(name="xp", bufs=12) as xp,
            tc.tile_pool(name="tp", bufs=3) as tp,
            tc.tile_pool(name="fevp", bufs=3) as fevp,
            tc.tile_pool(name="qtp", bufs=N_STILES + 1) as qtp,
            tc.tile_pool(name="outp", bufs=6) as outp,
            tc.tile_pool(name="stats", bufs=8) as stats,
            tc.tile_pool(name="ma7p", bufs=N_STILES + 1) as ma7p,
            tc.tile_pool(name="tpsum", bufs=2, space="PSUM") as tpsum,
            tc.tile_pool(name="mpsum", bufs=2, space="PSUM") as mpsum,
            tc.tile_pool(name="dramp", bufs=1, space="DRAM") as dramp,
        ):
            # ---------------- one-time setup ----------------
            identity = singles.tile([P, P], bf16)
            make_identity(nc, identity)
            identity_f = singles.tile([P, P], f32)
            make_identity(nc, identity_f)

            magneg = singles.tile([P, 1], f32)
            nc.vector.memset(magneg, -MAGIC)

            # signT8[i_sub, k, o] = ternarized sign of weight[o, k*128+i_sub]
            # fp8 for DoubleRow matmuls (values {-1,0,1}: exact)
            signT8 = singles.tile([P, N_KTILES, OUT_F], f8)
            alpha_raw = singles.tile([P, N_OTILES], f32)  # row sums of |w|

            wpool = _wstack.enter_context(tc.tile_pool(name="wpool", bufs=8))
            w_tiles = []
            for j in range(N_OTILES):
                w_t = wpool.tile([P, IN_F], f16, tag="w")
                w_tiles.append(w_t)

            def emit_wload(js):
                for j in js:
                    if j < N_OTILES:
                        nc.sync.dma_start(
                            out=w_tiles[j], in_=w_in[j * P : (j + 1) * P, :]
                        )

            def emit_wabs(j):
                # |w| row sums on DVE, interleaved into the quant stream
                nc.vector.tensor_reduce(
                    out=alpha_raw[:, j : j + 1],
                    in_=w_tiles[j],
                    axis=mybir.AxisListType.X,
                    op=Alu.add,
                    apply_absolute_value=True,
                )

            # ---- per-s-tile quant chain ----
            x_pairs = {}

            def emit_quant(s):
                # x rows arrive two s-tiles per 512 KiB DMA
                if s % 2 == 0:
                    x2 = xp.tile([P, 2, IN_F], f16, tag="x")
                    if s == 0:
                        # two single-tile DMAs: the s=0 chain starts as soon
                        # as the first 256 KiB lands (subtile deps)
                        for g in range(2):
                            nc.sync.dma_start(
                                out=x2[:, g, :],
                                in_=x_in[(s + g) * P : (s + g + 1) * P, :],
                            )
                    else:
                        nc.sync.dma_start(
                            out=x2,
                            in_=x_in[s * P : (s + 2) * P, :].rearrange(
                                "(two p) f -> p two f", p=P
                            ),
                        )
                    x_pairs[s] = x2
                    x_t = x2[:, 0, :]
                else:
                    x_t = x_pairs.pop(s - 1)[:, 1, :]
                ma = stats.tile([P, 1], f32, tag="ma")
                nc.vector.tensor_reduce(
                    out=ma,
                    in_=x_t,
                    axis=mybir.AxisListType.X,
                    op=Alu.max,
                    apply_absolute_value=True,
                )
                # row scale = max(ma, EPS)/7 ; inv = 7/max(ma, EPS)
                ma7 = ma7p.tile([P, 1], f32, tag="ma7")
                nc.vector.tensor_scalar(
                    out=ma7,
                    in0=ma,
                    scalar1=float(1.0 / 7.0),
                    scalar2=float(EPS / 7.0),
                    op0=Alu.mult,
                    op1=Alu.max,
                )
                inv = stats.tile([P, 1], f32, tag="inv")
                nc.vector.reciprocal(out=inv, in_=ma7)
                # t = x*inv + MAGIC (f32; fraction now rounded half-to-even)
                t_t = tp.tile([P, IN_F], f32, tag="t")
                nc.gpsimd.tensor_scalar(
                    out=t_t,
                    in0=x_t,
                    scalar1=inv,
                    scalar2=MAGIC,
                    op0=Alu.mult,
                    op1=Alu.add,
                )
                # transpose t into [i, s] layout via PE (8 blocks, one psum tile)
                qt_ps = tpsum.tile([P, IN_F], f32, tag="tps")
                for k in range(N_KTILES):
                    nc.tensor.transpose(
                        qt_ps[:, k * P : (k + 1) * P],
                        t_t[:, k * P : (k + 1) * P],
                        identity_f,
                    )
                # evict with fused -MAGIC subtract + fp8 cast (exact ints)
                qt_sb = qtp.tile([P, N_KTILES, P], f8, tag="qt")
                nc.scalar.activation(
                    out=qt_sb.rearrange("p k c -> p (k c)"),
                    in_=qt_ps,
                    func=Act.Identity,
                    bias=magneg,
                    scale=1.0,
                )
                return ma7, qt_sb

            out_pairs = {}

            def emit_matmul(s, ma7, qt_sb):
                # output rows leave two s-tiles per 1 MiB DMA
                if s % 2 == 0:
                    out2 = outp.tile([P, 2, OUT_F], f32, tag="o")
                    out_pairs[s] = out2
                    out_sb = out2[:, 0, :]
                else:
                    out2 = out_pairs[s - 1]
                    out_sb = out2[:, 1, :]
                ps = mpsum.tile([P, OUT_F], f32, tag="mm")
                for h in range(N_OHALF):
                    for t in range(N_KTILES // 2):
                        nc.tensor.matmul(
                            ps[:, h * MM_N : (h + 1) * MM_N],
                            lhsT=qt_sb[:, 2 * t : 2 * t + 2, :],
                            rhs=signT8[
                                :, 2 * t : 2 * t + 2, h * MM_N : (h + 1) * MM_N
                            ],
                            start=(t == 0),
                            stop=(t == N_KTILES // 2 - 1),
                            perf_mode=mybir.MatmulPerfMode.DoubleRow,
                        )
                # out = (S * rowscale) * colscale, split across two engines:
                # ACT evicts PSUM with the per-row scale (GPSIMD can't read
                # PSUM), Pool applies the per-column scale.
                fev = fevp.tile([P, OUT_F], f32, tag="fev")
                nc.scalar.activation(
                    out=fev, in_=ps, func=Act.Identity, scale=ma7
                )
                nc.gpsimd.tensor_tensor(
                    out=out_sb, in0=fev, in1=colb, op=Alu.mult
                )
                if with_bias:
                    nc.gpsimd.tensor_tensor(
                        out=out_sb, in0=out_sb, in1=biasb, op=Alu.add
                    )
                if s % 2 == 1:
                    nc.sync.dma_start(
                        out=out_d[(s - 1) * P : (s + 1) * P, :].rearrange(
                            "(two p) f -> p two f", p=P
                        ),
                        in_=out_pairs.pop(s - 1),
                    )

            def emit_wprep_tail():
                # global threshold = 0.05 * mean(|w|)
                g0 = stats.tile([P, 1], f32, tag="g0")
                nc.vector.tensor_reduce(
                    out=g0, in_=alpha_raw, axis=mybir.AxisListType.X, op=Alu.add
                )
                g1 = stats.tile([P, 1], f32, tag="g1")
                nc.gpsimd.partition_all_reduce(
                    out_ap=g1, in_ap=g0, channels=P, reduce_op=bass_isa.ReduceOp.add
                )
                nc.vector.tensor_scalar(
                    out=thr,
                    in0=g1,
                    scalar1=float(0.05 / (OUT_F * IN_F)),
                    scalar2=None,
                    op0=Alu.mult,
                )
                nc.vector.tensor_scalar(
                    out=nthr, in0=thr, scalar1=-1.0, scalar2=None, op0=Alu.mult
                )
                # alpha[o] = rowsum / IN_F
                nc.vector.tensor_scalar(
                    out=alpha_sb,
                    in0=alpha_raw,
                    scalar1=float(1.0 / IN_F),
                    scalar2=None,
                    op0=Alu.mult,
                )

                # ternary sign: sign = (w >= thr) + (w > -thr) - 1
                for j in range(N_OTILES):
                    tmp = wtmp.tile([P, IN_F], f16, tag="tmp")
                    nc.gpsimd.tensor_scalar(
                        out=tmp,
                        in0=w_tiles[j],
                        scalar1=nthr,
                        scalar2=-1.0,
                        op0=Alu.is_gt,
                        op1=Alu.add,
                    )
                    sgn = signp.tile([P, IN_F], bf16, tag="sgn")
                    nc.vector.scalar_tensor_tensor(
                        out=sgn,
                        in0=w_tiles[j],
                        scalar=thr,
                        in1=tmp,
                        op0=Alu.is_ge,
                        op1=Alu.add,
                    )
                    # transpose 8x [128,128] blocks into one PSUM bank, evict
                    ps = tpsum.tile([P, IN_F], bf16, tag="tps")
                    for k in range(N_KTILES):
                        nc.tensor.transpose(
                            ps[:, k * P : (k + 1) * P],
                            sgn[:, k * P : (k + 1) * P],
                            identity,
                        )
                    nc.scalar.activation(
                        out=signT8[:, :, j * P : (j + 1) * P],
                        in_=ps.rearrange("p (k c) -> p k c", k=N_KTILES),
                        func=Act.Copy,
                    )

                # column scale alpha broadcast to all partitions via DRAM bounce
                nc.sync.dma_start(
                    out=alpha_dram.rearrange("j p -> p j"), in_=alpha_sb
                )
                alpha_flat = alpha_dram.rearrange("j p -> (j p)")
                bcast_src = bass.AP(
                    tensor=alpha_flat.tensor,
                    offset=alpha_flat.offset,
                    ap=[[0, P]] + list(alpha_flat.ap),
                )
                nc.sync.dma_start(out=colb, in_=bcast_src)

                if with_bias:
                    bias_src = bass.AP(
                        tensor=b_in.tensor
                        if hasattr(b_in, "tensor")
                        else b_in[:].tensor,
                        offset=b_in[:].offset,
                        ap=[[0, P]] + list(b_in[:].ap),
                    )
                    nc.sync.dma_start(out=biasb, in_=bias_src)

            thr = singles.tile([P, 1], f32)
            nthr = singles.tile([P, 1], f32)
            alpha_sb = singles.tile([P, N_OTILES], f32)
            alpha_dram = dramp.tile([N_OTILES, P], f32)
            colb = singles.tile([P, OUT_F], f32)
            biasb = None
            if with_bias:
                biasb = singles.tile([P, OUT_F], f32, tag="biasb")

            # Phase 1: quantize + transpose s-tiles; |w| row-sums interleave
            # into the DVE stream early, the sign chain is emitted at WPREP_S,
            # and matmuls trail the quant chain by LEAD s-tiles.
            LEAD = min(int(os.environ.get("KLEAD", "6")), N_STILES)
            WPREP_S = min(N_OTILES // 2, N_STILES - 1)
            LEAD = max(LEAD, WPREP_S + 1)
            prologue = []
            for s in range(N_STILES):
                prologue.append(emit_quant(s))
                if s == 0:
                    emit_wload((0, 1, 2, 3))
                elif s == 1:
                    emit_wload((4, 5, 6, 7))
                for j in (2 * s, 2 * s + 1):
                    if j < N_OTILES:
                        emit_wabs(j)
                if s == WPREP_S:
                    emit_wprep_tail()
                    w_tiles.clear()
                    _wstack.close()  # releases the weight pool
                if s >= LEAD:
                    emit_matmul(s - LEAD, *prologue[s - LEAD])
            for s in range(max(0, N_STILES - LEAD), N_STILES):
                emit_matmul(s, *prologue[s])

    nc.compile()
    return nc


def _get_program(with_bias: bool):
    key = bool(with_bias)
    if key not in _prog_cache:
        _prog_cache[key] = _build_program(key)
    return _prog_cache[key]


def kernel(x: np.ndarray, weight: np.ndarray, bias: np.ndarray) -> np.ndarray:
    from concourse.bass_utils import run_bass_kernel_spmd

    B, S, in_f = x.shape
    out_f = weight.shape[0]
    assert in_f == IN_F and out_f == OUT_F and B * S == N_CORES * S_SHARD

    xf = np.ascontiguousarray(
        x.astype(np.float16, copy=False).reshape(-1, IN_F)
    )
    w = np.ascontiguousarray(weight.astype(np.float16, copy=False))
    b = np.ascontiguousarray(bias.astype(np.float32, copy=False))

    with_bias = bool(np.any(b != 0.0))
    nc = _get_program(with_bias)

    in_maps = []
    for c in range(N_CORES):
        m = {
            "x_shard": xf[c * S_SHARD : (c + 1) * S_SHARD],
            "weight": w,
        }
        if with_bias:
            m["bias"] = b
        in_maps.append(m)

    res = run_bass_kernel_spmd(nc, in_maps, core_ids=list(range(N_CORES)))
    out = np.concatenate([res.results[c]["out"] for c in range(N_CORES)], axis=0)
    return out.reshape(B, S, OUT_F).astype(np.float32, copy=False)


# revision 5
# speedup vs baseline: 1.2275x; 1.2275x over previous
"""BitLinear (4-bit activation quant + ternary weight) Trainium2 kernel.

Full computation:
    xq  = round(clip(x / max_abs(x, row) * 7)) * max_abs / 7      (per-row 4-bit quant)
    wq  = sign_thresholded(w) * mean_abs(w, row)                   (ternary weight)
    out = xq @ wq.T + bias

Strategy (8 NeuronCores, data-parallel over rows of x):
  - Shard x rows 8 ways; replicate weight.
  - x and weight ship to the device as f16 (halves the HBM read traffic; the
    quant decisions from f16 inputs keep end-to-end rel err ~1e-2, inside the
    2e-2 gate). Output is exact f32 scaling of integer PE accumulations.
  - Matmul runs on exact small integers in fp8 (q in [-8,7], sign in
    {-1,0,1}) with DoubleRow perf mode. Rounding uses the +1.5*2^23
    magic-number trick == round-half-even.
  - Engine balance per 128-row s-tile (steady state): DVE absmax+scales and
    most column-scale multiplies; Pool the magic multiply-add and 1/3 of the
    column-scale multiplies; ACT both PSUM evictions (qt and out*rowscale)
    plus out-store DMA issues; PE transposes + matmuls at full clock.
"""

import os
import sys

os.environ.setdefault("MYCRO_LOCAL_CACHE", "1")

for _p in ("/opt/trn_rl_repo", "/root/.axon_site/_ro/trn_rl_repo"):
    if os.path.isdir(_p) and _p not in sys.path:
        sys.path.insert(0, _p)

import numpy as np

N_CORES = 8
S_SHARD = 4096
IN_F = 1024
OUT_F = 1024
P = 128
N_STILES = S_SHARD // P  # 32
N_KTILES = IN_F // P  # 8
N_OTILES = OUT_F // P  # 8
MM_N = 512
N_OHALF = OUT_F // MM_N  # 2
OUT_B = 4  # s-tiles per output store (2 MiB transfers)

MAGIC = 12582912.0
EPS = 1e-06

_prog_cache = {}


def _build_program(with_bias: bool):
    import concourse.bass as bass
    import concourse.mybir as mybir
    import concourse.tile as tile
    from concourse import bacc, bass_isa
    from concourse.masks import make_identity

    f32 = mybir.dt.float32
    f16 = mybir.dt.float16
    bf16 = mybir.dt.bfloat16
    f8 = mybir.dt.float8e4
    Alu = mybir.AluOpType
    Act = mybir.ActivationFunctionType

    nc = bacc.Bacc("TRN2", target_bir_lowering=False, debug=False)

    x_in = nc.dram_tensor("x_shard", [S_SHARD, IN_F], f16, kind="ExternalInput")
    w_in = nc.dram_tensor("weight", [OUT_F, IN_F], f16, kind="ExternalInput")
    if with_bias:
        b_in = nc.dram_tensor("bias", [OUT_F], f32, kind="ExternalInput")
    out_d = nc.dram_tensor("out", [S_SHARD, OUT_F], f32, kind="ExternalOutput")

    WPREP_S = int(os.environ.get("KWPREP", "8"))
    SIGN_PER_S = int(os.environ.get("KSIGNPS", "2"))
    LEAD = int(os.environ.get("KLEAD", "13"))
    XBUFS = int(os.environ.get("KXBUFS", "10"))
    OBUFS = int(os.environ.get("KOBUFS", "4"))

    with tile.TileContext(nc) as tc:
        from contextlib import ExitStack as _ES

        _wstack = _ES()
        with (
            tc.tile_pool(name="singles", bufs=1) as singles,
            tc.tile_pool(name="wtmp", bufs=2) as wtmp,
            tc.tile_pool(name="signp", bufs=2) as signp,
            tc.tile_pool(name="xp", bufs=XBUFS) as xp,
            tc.tile_pool(name="tp", bufs=3) as tp,
            tc.tile_pool(name="fevp", bufs=3) as fevp,
            tc.tile_pool(name="qtp", bufs=N_STILES + 1) as qtp,
            tc.tile_pool(name="outp", bufs=OBUFS) as outp,
            tc.tile_pool(name="stats", bufs=8) as stats,
            tc.tile_pool(name="ma7p", bufs=N_STILES + 1) as ma7p,
            tc.tile_pool(name="tpsum", bufs=2, space="PSUM") as tpsum,
            tc.tile_pool(name="mpsum", bufs=2, space="PSUM") as mpsum,
            tc.tile_pool(name="dramp", bufs=1, space="DRAM") as dramp,
        ):
            identity = singles.tile([P, P], bf16)
            make_identity(nc, identity)
            identity_f = singles.tile([P, P], f32)
            make_identity(nc, identity_f)

            magneg = singles.tile([P, 1], f32)
            nc.vector.memset(magneg, -MAGIC)

            signT8 = singles.tile([P, N_KTILES, OUT_F], f8)
            alpha_raw = singles.tile([P, N_OTILES], f32)

            wpool = _wstack.enter_context(tc.tile_pool(name="wpool", bufs=8))
            w_tiles = []
            for j in range(N_OTILES):
                w_t = wpool.tile([P, IN_F], f16, tag="w")
                w_tiles.append(w_t)

            def emit_wload(js):
                for j in js:
                    if j < N_OTILES:
                        nc.sync.dma_start(
                            out=w_tiles[j], in_=w_in[j * P : (j + 1) * P, :]
                        )

            def emit_wabs(j):
                nc.vector.tensor_reduce(
                    out=alpha_raw[:, j : j + 1],
                    in_=w_tiles[j],
                    axis=mybir.AxisListType.X,
                    op=Alu.add,
                    apply_absolute_value=True,
                )

            x_pairs = {}

            def emit_quant(s):
                if s % 2 == 0:
                    x2 = xp.tile([P, 2, IN_F], f16, tag="x")
                    if s == 0:
                        for g in range(2):
                            nc.sync.dma_start(
                                out=x2[:, g, :],
                                in_=x_in[(s + g) * P : (s + g + 1) * P, :],
                            )
                    else:
                        nc.sync.dma_start(
                            out=x2,
                            in_=x_in[s * P : (s + 2) * P, :].rearrange(
                                "(two p) f -> p two f", p=P
                            ),
                        )
                    x_pairs[s] = x2
                    x_t = x2[:, 0, :]
                else:
                    x_t = x_pairs.pop(s - 1)[:, 1, :]
                ma = stats.tile([P, 1], f32, tag="ma")
                nc.vector.tensor_reduce(
                    out=ma,
                    in_=x_t,
                    axis=mybir.AxisListType.X,
                    op=Alu.max,
                    apply_absolute_value=True,
                )
                ma7 = ma7p.tile([P, 1], f32, tag="ma7")
                nc.vector.tensor_scalar(
                    out=ma7,
                    in0=ma,
                    scalar1=float(1.0 / 7.0),
                    scalar2=float(EPS / 7.0),
                    op0=Alu.mult,
                    op1=Alu.max,
                )
                inv = stats.tile([P, 1], f32, tag="inv")
                nc.vector.reciprocal(out=inv, in_=ma7)
                # t = x*inv + MAGIC (f32; fraction now rounded half-to-even)
                t_t = tp.tile([P, IN_F], f32, tag="t")
                nc.gpsimd.tensor_scalar(
                    out=t_t,
                    in0=x_t,
                    scalar1=inv,
                    scalar2=MAGIC,
                    op0=Alu.mult,
                    op1=Alu.add,
                )
                qt_ps = tpsum.tile([P, IN_F], f32, tag="tps")
                for k in range(N_KTILES):
                    nc.tensor.transpose(
                        qt_ps[:, k * P : (k + 1) * P],
                        t_t[:, k * P : (k + 1) * P],
                        identity_f,
                    )
                qt_sb = qtp.tile([P, N_KTILES, P], f8, tag="qt")
                nc.scalar.activation(
                    out=qt_sb.rearrange("p k c -> p (k c)"),
                    in_=qt_ps,
                    func=Act.Identity,
                    bias=magneg,
                    scale=1.0,
                )
                return ma7, qt_sb

            out_blocks = {}

            def emit_matmul(s, ma7, qt_sb):
                sb = (s // OUT_B) * OUT_B
                if s % OUT_B == 0:
                    ob = outp.tile([P, OUT_B, OUT_F], f32, tag="o")
                    out_blocks[sb] = ob
                else:
                    ob = out_blocks[sb]
                out_sb = ob[:, s % OUT_B, :]
                ps = mpsum.tile([P, OUT_F], f32, tag="mm")
                for h in range(N_OHALF):
                    for t in range(N_KTILES // 2):
                        nc.tensor.matmul(
                            ps[:, h * MM_N : (h + 1) * MM_N],
                            lhsT=qt_sb[:, 2 * t : 2 * t + 2, :],
                            rhs=signT8[
                                :, 2 * t : 2 * t + 2, h * MM_N : (h + 1) * MM_N
                            ],
                            start=(t == 0),
                            stop=(t == N_KTILES // 2 - 1),
                            perf_mode=mybir.MatmulPerfMode.DoubleRow,
                        )
                # out = (S * rowscale) * colscale: ACT evicts PSUM with the
                # per-row scale (GPSIMD can't read PSUM), then the per-column
                # multiply runs on DVE (2 of 3 tiles) or Pool (1 of 3).
                fev = fevp.tile([P, OUT_F], f32, tag="fev")
                nc.scalar.activation(
                    out=fev, in_=ps, func=Act.Identity, scale=ma7
                )
                eng = nc.gpsimd if (s % 3 == 0) else nc.vector
                eng.tensor_tensor(out=out_sb, in0=fev, in1=colb, op=Alu.mult)
                if with_bias:
                    nc.gpsimd.tensor_tensor(
                        out=out_sb, in0=out_sb, in1=biasb, op=Alu.add
                    )
                if s % OUT_B == OUT_B - 1:
                    nc.scalar.dma_start(
                        out=out_d[sb * P : (s + 1) * P, :].rearrange(
                            "(b p) f -> p b f", p=P
                        ),
                        in_=out_blocks.pop(sb),
                    )

            def emit_wprep_head():
                # global threshold = 0.05 * mean(|w|)
                g0 = stats.tile([P, 1], f32, tag="g0")
                nc.vector.tensor_reduce(
                    out=g0, in_=alpha_raw, axis=mybir.AxisListType.X, op=Alu.add
                )
                g1 = stats.tile([P, 1], f32, tag="g1")
                nc.gpsimd.partition_all_reduce(
                    out_ap=g1, in_ap=g0, channels=P, reduce_op=bass_isa.ReduceOp.add
                )
                nc.vector.tensor_scalar(
                    out=thr,
                    in0=g1,
                    scalar1=float(0.05 / (OUT_F * IN_F)),
                    scalar2=None,
                    op0=Alu.mult,
                )
                nc.vector.tensor_scalar(
                    out=nthr, in0=thr, scalar1=-1.0, scalar2=None, op0=Alu.mult
                )
                nc.vector.tensor_scalar(
                    out=alpha_sb,
                    in0=alpha_raw,
                    scalar1=float(1.0 / IN_F),
                    scalar2=None,
                    op0=Alu.mult,
                )
                # column scale alpha broadcast to all partitions via DRAM bounce
                nc.sync.dma_start(
                    out=alpha_dram.rearrange("j p -> p j"), in_=alpha_sb
                )
                alpha_flat = alpha_dram.rearrange("j p -> (j p)")
                bcast_src = bass.AP(
                    tensor=alpha_flat.tensor,
                    offset=alpha_flat.offset,
                    ap=[[0, P]] + list(alpha_flat.ap),
                )
                nc.sync.dma_start(out=colb, in_=bcast_src)
                if with_bias:
                    bias_src = bass.AP(
                        tensor=b_in.tensor
                        if hasattr(b_in, "tensor")
                        else b_in[:].tensor,
                        offset=b_in[:].offset,
                        ap=[[0, P]] + list(b_in[:].ap),
                    )
                    nc.sync.dma_start(out=biasb, in_=bias_src)

            def emit_sign(j):
                # ternary sign: sign = (w >= thr) + (w > -thr) - 1
                tmp = wtmp.tile([P, IN_F], f16, tag="tmp")
                nc.gpsimd.tensor_scalar(
                    out=tmp,
                    in0=w_tiles[j],
                    scalar1=nthr,
                    scalar2=-1.0,
                    op0=Alu.is_gt,
                    op1=Alu.add,
                )
                sgn = signp.tile([P, IN_F], bf16, tag="sgn")
                nc.vector.scalar_tensor_tensor(
                    out=sgn,
                    in0=w_tiles[j],
                    scalar=thr,
                    in1=tmp,
                    op0=Alu.is_ge,
                    op1=Alu.add,
                )
                ps = tpsum.tile([P, IN_F], bf16, tag="tps")
                for k in range(N_KTILES):
                    nc.tensor.transpose(
                        ps[:, k * P : (k + 1) * P],
                        sgn[:, k * P : (k + 1) * P],
                        identity,
                    )
                nc.scalar.activation(
                    out=signT8[:, :, j * P : (j + 1) * P],
                    in_=ps.rearrange("p (k c) -> p k c", k=N_KTILES),
                    func=Act.Copy,
                )

            thr = singles.tile([P, 1], f32)
            nthr = singles.tile([P, 1], f32)
            alpha_sb = singles.tile([P, N_OTILES], f32)
            alpha_dram = dramp.tile([N_OTILES, P], f32)
            colb = singles.tile([P, OUT_F], f32)
            biasb = None
            if with_bias:
                biasb = singles.tile([P, OUT_F], f32, tag="biasb")

            sign_emitted = 0
            for s in range(N_STILES):
                prologue_item = emit_quant(s)
                if s == 0:
                    emit_wload((0, 1, 2, 3))
                    prologue = []
                elif s == 1:
                    emit_wload((4, 5, 6, 7))
                prologue.append(prologue_item)
                # one |w| row-sum per s-tile, s=0..7
                if s < N_OTILES:
                    emit_wabs(s)
                if s == WPREP_S:
                    emit_wprep_head()
                if s >= WPREP_S and sign_emitted < N_OTILES:
                    for _ in range(SIGN_PER_S):
                        if sign_emitted < N_OTILES:
                            emit_sign(sign_emitted)
                            sign_emitted += 1
                    if sign_emitted == N_OTILES:
                        w_tiles.clear()
                        _wstack.close()
                if s >= LEAD:
                    emit_matmul(s - LEAD, *prologue[s - LEAD])
            for s in range(max(0, N_STILES - LEAD), N_STILES):
                emit_matmul(s, *prologue[s])

    nc.compile()
    return nc


def _get_program(with_bias: bool):
    key = bool(with_bias)
    if key not in _prog_cache:
        _prog_cache[key] = _build_program(key)
    return _prog_cache[key]


def kernel(x: np.ndarray, weight: np.ndarray, bias: np.ndarray) -> np.ndarray:
    from concourse.bass_utils import run_bass_kernel_spmd

    B, S, in_f = x.shape
    out_f = weight.shape[0]
    assert in_f == IN_F and out_f == OUT_F and B * S == N_CORES * S_SHARD

    xf = np.ascontiguousarray(
        x.astype(np.float16, copy=False).reshape(-1, IN_F)
    )
    w = np.ascontiguousarray(weight.astype(np.float16, copy=False))
    b = np.ascontiguousarray(bias.astype(np.float32, copy=False))

    with_bias = bool(np.any(b != 0.0))
    nc = _get_program(with_bias)

    in_maps = []
    for c in range(N_CORES):
        m = {
            "x_shard": xf[c * S_SHARD : (c + 1) * S_SHARD],
            "weight": w,
        }
        if with_bias:
            m["bias"] = b
        in_maps.append(m)

    res = run_bass_kernel_spmd(nc, in_maps, core_ids=list(range(N_CORES)))
    out = np.concatenate([res.results[c]["out"] for c in range(N_CORES)], axis=0)
    return out.reshape(B, S, OUT_F).astype(np.float32, copy=False)
